# revision 46
# baseline (speedup 1.0000x reference)
"""Multi-head attention (B=4, N=2048, D=1024, H=16) on 8 TRN2 NeuronCores.

Sharding: core c = (batch b = c // 2, head-group hg = c % 2). Each core:
  - computes Q/K/V for its 8 heads (tensor-parallel slice of qkv_w),
  - runs attention for those heads,
  - computes a partial output projection against its 512 columns of proj_w.
Host sums the two partials per batch and adds biases folded on the host.

Device layouts (all feature-on-partition so that scores come out as
S^T [k, q] with k on partitions — no transposes anywhere):
  xt  [128, 8, 2048]  bf16 : x[b]^T, d = kt*128 + p
  wqk [128, 8, 1024]  bf16 : lhsT for Q (slots 0..3) and K (slots 4..7);
                             slot t covers the head pair (2t, 2t+1), so the
                             PSUM partition j of an output tile = head
                             (2t + j//64), hd = j % 64.
  wv  [128, 8, 512]   bf16 : rhs for V (token-on-partition orientation)
  wp  [128, 4, 1024]  bf16 : lhsT-side contraction layout for the proj
  bqk [128, 8]        f32  : per-feature q/k bias (zero in practice)
  out [2048, 1024]    f32  : partial projection output

Attention per head: S^T = K @ Q^T (k on partitions), P = exp(S^T/8) on ACT
(PSUM->SBUF, bf16), O^T_aug = V_aug.T @ P where V_aug = [V | 1] gives the
softmax denominator as row 64 (in-band).  Normalization multiplies by a
reciprocal broadcast across partitions via a DRAM round-trip DMA.

Softmax skips max-subtraction (scores ~N(0,1); exp never overflows fp32).
V-bias and proj bias are folded host-side: softmax rows sum to 1, so the
V bias contributes exactly proj_w @ v_bias to every output row.

Pair t+1's Q/K projection matmuls are interleaved into pair t's attention
loop to keep the TensorE dense (HAM stays at 2.4 GHz) and hide the QKV
phase inside the ACT-bound attention phase.
"""

import numpy as np
import ml_dtypes

import concourse.bass as bass
import concourse.tile as tile
from concourse import bacc, mybir
from concourse._compat import with_exitstack

B, N, D, H, HD = 4, 2048, 1024, 16, 64
NCORES = 8
HPC = 8          # heads per core
FPC = HPC * HD   # 512 features per core
KT = 8           # d-contraction tiles of 128
KTT = 16         # key-token tiles of 128
QB = 512         # q-block size
NQB = N // QB
SCALE = HD ** -0.5

F32 = mybir.dt.float32
BF16 = mybir.dt.bfloat16
EXP = mybir.ActivationFunctionType.Exp


@with_exitstack
def _attn_body(ctx, tc, xt_d, wqk_d, wv_d, wp_d, bqk_d, out_d):
    nc = tc.nc

    singles = ctx.enter_context(tc.tile_pool(name="singles", bufs=1))
    evac = ctx.enter_context(tc.tile_pool(name="evac", bufs=4))
    ppool = ctx.enter_context(tc.tile_pool(name="ppool", bufs=4))
    rpool = ctx.enter_context(tc.tile_pool(name="rpool", bufs=4))
    dpool = ctx.enter_context(tc.tile_pool(name="dpool", bufs=8, space="DRAM"))
    ps_s = ctx.enter_context(tc.tile_pool(name="ps_s", bufs=2, space="PSUM"))
    ps_av = ctx.enter_context(tc.tile_pool(name="ps_av", bufs=3, space="PSUM"))
    ps_w = ctx.enter_context(tc.tile_pool(name="ps_w", bufs=1, space="PSUM"))

    # Resident SBUF tensors.  Weights first, then x in token-range chunks so
    # the V projection can start before the full input has landed; wp last
    # (only the projection needs it).
    wv_sb = singles.tile([128, KT, FPC], BF16)
    nc.sync.dma_start(wv_sb, wv_d[:])
    wqk_sb = singles.tile([128, KT, 2 * FPC], BF16)
    nc.sync.dma_start(wqk_sb, wqk_d[:])
    bqk_sb = singles.tile([128, 8], F32)
    nc.sync.dma_start(bqk_sb, bqk_d[:])
    xt_sb = singles.tile([128, KT, N], BF16)
    for c in range(8):
        nc.sync.dma_start(xt_sb[:, :, c * 256:(c + 1) * 256],
                          xt_d[:, :, c * 256:(c + 1) * 256])
    wp_sb = singles.tile([128, 4, D], BF16)
    nc.sync.dma_start(wp_sb, wp_d[:])

    qk_sb = singles.tile([128, 8, N], BF16)          # Q^T slots 0..3, K^T slots 4..7
    v_sb = singles.tile([128, KTT, HPC, HD + 1], BF16)  # V_aug, token-on-partition
    o_sb = singles.tile([128, 4, N], BF16)           # normalized attn out, f-on-part
    nc.vector.memset(v_sb[:, :, :, HD], 1.0)         # the ones column

    def emit_qk(ft, qt, pool, tag):
        """One (ft, qt) group of the Q/K projection: 8 matmuls + bias evac."""
        ps = pool.tile([128, 512], F32, tag=tag, name="qk_ps")
        for kt in range(KT):
            nc.tensor.matmul(
                ps,
                wqk_sb[:, kt, ft * 128:(ft + 1) * 128],
                xt_sb[:, kt, qt * 512:(qt + 1) * 512],
                start=(kt == 0), stop=(kt == KT - 1),
            )
        nc.vector.tensor_scalar_add(
            qk_sb[:, ft, qt * 512:(qt + 1) * 512], ps, bqk_sb[:, ft:ft + 1])

    def emit_v(mt, pool, tag):
        """One token-tile of the V projection: 8 matmuls + strided evac."""
        ps = pool.tile([128, 512], F32, tag=tag, name="v_ps")
        for kt in range(KT):
            nc.tensor.matmul(
                ps,
                xt_sb[:, kt, mt * 128:(mt + 1) * 128],
                wv_sb[:, kt, :],
                start=(kt == 0), stop=(kt == KT - 1),
            )
        nc.vector.tensor_copy(
            v_sb[:, mt, :, 0:HD], ps.rearrange("p (h e) -> p h e", h=HPC))

    def emit_proj(mt, et, pool, tag, on_act=False):
        """One output-projection group: 4 matmuls + evac + store."""
        ps = pool.tile([128, 512], F32, tag=tag, name="pj_ps")
        for t4 in range(4):
            nc.tensor.matmul(
                ps,
                o_sb[:, t4, mt * 128:(mt + 1) * 128],
                wp_sb[:, t4, et * 512:(et + 1) * 512],
                start=(t4 == 0), stop=(t4 == 3),
            )
        ot = evac.tile([128, 512], F32, tag="oevac", name="o_evac")
        if on_act:  # ACT is idle in the tail; DVE may be behind a reciprocal
            nc.scalar.copy(ot, ps)
        else:
            nc.vector.tensor_copy(ot, ps)
        nc.sync.dma_start(
            out_d[mt * 128:(mt + 1) * 128, et * 512:(et + 1) * 512], ot)

    # normalize: o = av * (1/denom); denom = row 64 of avs (ones col).
    # DVE reciprocal runs at 1/8 rate, so a single-partition [1,1024]
    # reciprocal costs ~6.5us: bounce the denominators through DRAM into a
    # [128, 8] layout so the reciprocal uses all 128 lanes (~70ns), and read
    # the result back as a partition-broadcast.  Stage 2 (reciprocal + the
    # multiplies) is deferred one block so the DVE never sits waiting on the
    # DMA chain and blocking the rest of its in-order queue; the multiplies
    # run on the otherwise-idle GpSimd engine, where waiting is harmless.
    def normalize_stage1(t, qb, avs):
        rd = dpool.tile([2, QB], F32, name="d_dram")
        nc.sync.dma_start(rd[0:1, :], avs[HD:HD + 1, 0:QB])
        nc.sync.dma_start(rd[1:2, :], avs[HD:HD + 1, QB:2 * QB])
        d128 = rpool.tile([128, 8], F32, tag="d128", name="d128_t")
        nc.sync.dma_start(
            d128, rd[:].rearrange("two (a p) -> p (two a)", p=128))
        return (t, qb, avs, d128)

    def normalize_stage2(st):
        t, qb, avs, d128 = st
        q0 = qb * QB
        r128 = rpool.tile([128, 8], F32, tag="r128", name="r128_t")
        nc.vector.reciprocal(r128, d128)
        rr = dpool.tile([2, QB], F32, name="r_dram")
        nc.sync.dma_start(
            rr[:].rearrange("two (a p) -> p (two a)", p=128), r128)
        rb = rpool.tile([64, 2 * QB], F32, tag="rb", name="rb_t")
        nc.sync.dma_start(rb[:, 0:QB], rr[0:1, :].partition_broadcast(64))
        nc.sync.dma_start(rb[:, QB:2 * QB],
                          rr[1:2, :].partition_broadcast(64))
        nc.gpsimd.tensor_mul(o_sb[0:64, t, q0:q0 + QB], avs[0:HD, 0:QB],
                             rb[:, 0:QB])
        ot = rpool.tile([64, QB], BF16, tag="otmp", name="o_tmp")
        nc.gpsimd.tensor_mul(ot, avs[0:HD, QB:2 * QB], rb[:, QB:2 * QB])
        nc.gpsimd.dma_start(o_sb[64:128, t, q0:q0 + QB], ot)

    def normalize_direct(t, qb, avs):
        """Pair-3 variant: plain DVE reciprocal (6.5us busy but no DMA
        chain), so o_sb is ready quickly and the projection can stream in
        behind it.  DVE has ~9us of slack per 18us block here (no QK
        evacuations left), so the slow reciprocal just fills idle DVE time
        without delaying the next block's AV evacuations."""
        q0 = qb * QB
        r = rpool.tile([HD + 1, 2 * QB], F32, tag="rdir", name="r_dir")
        nc.vector.reciprocal(r[HD:HD + 1, :], avs[HD:HD + 1, :])
        rb = rpool.tile([64, 2 * QB], F32, tag="rb", name="rb_t")
        rd = dpool.tile([2, QB], F32, name="rdd")
        nc.sync.dma_start(rd[0:1, :], r[HD:HD + 1, 0:QB])
        nc.sync.dma_start(rd[1:2, :], r[HD:HD + 1, QB:2 * QB])
        nc.sync.dma_start(rb[:, 0:QB], rd[0:1, :].partition_broadcast(64))
        nc.sync.dma_start(rb[:, QB:2 * QB], rd[1:2, :].partition_broadcast(64))
        nc.vector.tensor_mul(o_sb[0:64, t, q0:q0 + QB], avs[0:HD, 0:QB],
                             rb[:, 0:QB])
        ot = rpool.tile([64, QB], BF16, tag="otmp", name="o_tmp")
        nc.vector.tensor_mul(ot, avs[0:HD, QB:2 * QB], rb[:, QB:2 * QB])
        nc.gpsimd.dma_start(o_sb[64:128, t, q0:q0 + QB], ot)

    # ---- Prologue: only what pair-0/qb-0 needs soon ----
    # V token-tiles 0..11 (AV(kt) consumes V(mt=kt) one per cycle), K(0,qt)
    # for qt 0,1 (scores kt 0..7) and Q(0,qt=0).
    for mt in range(12):
        emit_v(mt, ps_av, "psAV")
    emit_qk(4, 0, ps_av, "psAV")
    emit_qk(0, 0, ps_av, "psAV")
    emit_qk(4, 1, ps_av, "psAV")

    # Deadline-ordered feeds through the single ps_w slot, rate-limited so
    # the PE never falls behind ACT: pair 0 pops one group every 2 cycles,
    # pairs 1/2 every 4 (8-matmul groups), pair 3 every 2 (4-matmul proj
    # groups, appended as each q-block's normalize lands).
    feeds = [
        [("qk", 4, 2), ("qk", 0, 1), ("qk", 4, 3), ("v", 12), ("v", 13),
         ("v", 14), ("v", 15), ("qk", 0, 2), ("qk", 0, 3)]
        + [("qk", f, qt) for qt in range(4) for f in (1, 5)],
        [("qk", f, qt) for qt in range(4) for f in (2, 6)],
        [("qk", f, qt) for qt in range(4) for f in (3, 7)],
        [],  # pair 3: proj groups appended as q-blocks complete
    ]
    cadence = [2, 4, 4, 2]

    # ---- Attention: one flat software pipeline over (t, qb, kt).
    # AV runs one cycle behind scores/exp so ACT never waits on the PE; each
    # block's normalize is emitted right after its last AV (one cycle into
    # the next block).
    blocks = [(t, qb) for t in range(4) for qb in range(NQB)]
    pending = None  # (t, qb, kt, av_e, av_o, pt, start, fin)
    norm_q = []     # deferred normalize stage-2 states

    def flush_pending():
        nonlocal pending
        if pending is None:
            return
        t, qb, kt, av_e, av_o, pt, st, fin = pending
        nc.tensor.matmul(av_e, v_sb[:, kt, 2 * t, :], pt[:, 0:512],
                         start=st, stop=fin)
        nc.tensor.matmul(av_o, v_sb[:, kt, 2 * t + 1, :],
                         pt[:, 512:1024], start=st, stop=fin)
        if fin:
            avs = rpool.tile([HD + 1, 2 * QB], F32, tag="avs", name="avs_t")
            nc.vector.tensor_copy(avs[:, 0:QB], av_e)
            nc.vector.tensor_copy(avs[:, QB:2 * QB], av_o)
            if t < 3:
                if norm_q:
                    normalize_stage2(norm_q.pop(0))
                norm_q.append(normalize_stage1(t, qb, avs))
            else:
                # pair 3: direct normalize; proj for the PREVIOUS q-block
                # becomes feedable now (its o_sb landed a block ago).
                if qb > 0:
                    feeds[3].extend(
                        [("proj", mt, et)
                         for mt in range(4 * (qb - 1), 4 * qb)
                         for et in range(2)])
                normalize_direct(t, qb, avs)
        pending = None

    for t, qb in blocks:
        it = qb * KTT
        q0 = qb * QB
        av_e = ps_av.tile([HD + 1, QB], F32, tag="psAV", name="av_e")
        av_o = ps_av.tile([HD + 1, QB], F32, tag="psAV", name="av_o")
        for kt in range(KTT):
            k0 = kt * 128
            sp = ps_s.tile([128, 1024], F32, tag="psS", name="s_ps")
            # scores S^T for the pair: even head rows 0:64, odd 64:128
            nc.tensor.matmul(
                sp[:, 0:512],
                qk_sb[0:64, 4 + t, k0:k0 + 128],
                qk_sb[0:64, t, q0:q0 + 512],
                start=True, stop=True,
            )
            nc.tensor.matmul(
                sp[:, 512:1024],
                qk_sb[64:128, 4 + t, k0:k0 + 128],
                qk_sb[64:128, t, q0:q0 + 512],
                start=True, stop=True,
            )
            pt = ppool.tile([128, 1024], BF16, tag="pt", name="p_t")
            nc.scalar.activation(pt, sp, EXP, scale=SCALE)
            flush_pending()
            pending = (t, qb, kt, av_e, av_o, pt, kt == 0, kt == KTT - 1)
            if feeds[t] and (qb * KTT + kt) % cadence[t] == 0:
                g = feeds[t].pop(0)
                if g[0] == "qk":
                    emit_qk(g[1], g[2], ps_w, "psW")
                elif g[0] == "v":
                    emit_v(g[1], ps_w, "psW")
                else:
                    emit_proj(g[1], g[2], ps_w, "psW")
    flush_pending()
    while norm_q:
        normalize_stage2(norm_q.pop(0))

    # ---- Tail: remaining projection groups (q-block 2 leftovers via the
    # feed list, then q-block 3's) ----
    for g in feeds[3]:
        emit_proj(g[1], g[2], ps_av, "psAV", on_act=True)
    for mt in range(12, KTT):
        for et in range(2):
            emit_proj(mt, et, ps_av, "psAV", on_act=True)


def build_nc():
    nc = bacc.Bacc()
    xt = nc.declare_dram_parameter("xt", [128, KT, N], BF16, isOutput=False)
    wqk = nc.declare_dram_parameter("wqk", [128, KT, 2 * FPC], BF16, isOutput=False)
    wv = nc.declare_dram_parameter("wv", [128, KT, FPC], BF16, isOutput=False)
    wp = nc.declare_dram_parameter("wp", [128, 4, D], BF16, isOutput=False)
    bqk = nc.declare_dram_parameter("bqk", [128, 8], F32, isOutput=False)
    out = nc.declare_dram_parameter("out", [N, D], F32, isOutput=True)
    with tile.TileContext(nc) as tc:
        _attn_body(tc, xt, wqk, wv, wp, bqk, out)
    nc.finalize()
    return nc


BF = ml_dtypes.bfloat16


def prep_core_inputs(x, qkv_w, qkv_b, proj_w, c):
    """Build the per-core input map (numpy, final SBUF layouts)."""
    b, hg = divmod(c, 2)
    f0 = hg * FPC
    xt = np.ascontiguousarray(x[b].T)                     # [1024, 2048] f32
    xt_sb = xt.reshape(KT, 128, N).transpose(1, 0, 2)     # [128, 8, 2048]
    wq = qkv_w[f0:f0 + FPC]
    wk = qkv_w[D + f0:D + f0 + FPC]
    wqk = np.concatenate([wq, wk], axis=0)                # [1024, 1024]
    wqk_sb = wqk.T.reshape(KT, 128, 2 * FPC).transpose(1, 0, 2)
    wv = qkv_w[2 * D + f0:2 * D + f0 + FPC]               # [512, 1024]
    wv_sb = wv.T.reshape(KT, 128, FPC).transpose(1, 0, 2)
    wp = proj_w[:, f0:f0 + FPC]                           # [1024e, 512f]
    wp_sb = wp.T.reshape(4, 128, D).transpose(1, 0, 2)
    bqk = np.concatenate(
        [qkv_b[f0:f0 + FPC], qkv_b[D + f0:D + f0 + FPC]]).reshape(8, 128).T
    return {
        "xt": np.ascontiguousarray(xt_sb).astype(BF),
        "wqk": np.ascontiguousarray(wqk_sb).astype(BF),
        "wv": np.ascontiguousarray(wv_sb).astype(BF),
        "wp": np.ascontiguousarray(wp_sb).astype(BF),
        "bqk": np.ascontiguousarray(bqk).astype(np.float32),
    }


def expected_core_out(x, qkv_w, qkv_b, proj_w, c):
    """Numpy model of one core's partial output (for sim debugging)."""
    b, hg = divmod(c, 2)
    f0 = hg * FPC
    xb = x[b].astype(np.float32)
    q = xb @ qkv_w[f0:f0 + FPC].T + qkv_b[f0:f0 + FPC]
    k = xb @ qkv_w[D + f0:D + f0 + FPC].T + qkv_b[D + f0:D + f0 + FPC]
    v = xb @ qkv_w[2 * D + f0:2 * D + f0 + FPC].T          # v-bias folded on host
    out = np.zeros((N, D), np.float32)
    for h in range(HPC):
        qs = q[:, h * HD:(h + 1) * HD]
        ks = k[:, h * HD:(h + 1) * HD]
        vs = v[:, h * HD:(h + 1) * HD]
        s = (qs @ ks.T) * SCALE
        p = np.exp(s - s.max(axis=1, keepdims=True))
        p /= p.sum(axis=1, keepdims=True)
        out += (p @ vs) @ proj_w[:, f0 + h * HD:f0 + (h + 1) * HD].T
    return out


_NC_CACHE = {}


def kernel(x, qkv_w, qkv_b, proj_w, proj_b):
    from concourse.bass_utils import run_bass_kernel_spmd

    x = np.asarray(x, dtype=np.float32)
    qkv_w = np.asarray(qkv_w, dtype=np.float32)
    qkv_b = np.asarray(qkv_b, dtype=np.float32)
    proj_w = np.asarray(proj_w, dtype=np.float32)
    proj_b = np.asarray(proj_b, dtype=np.float32)

    if "nc" not in _NC_CACHE:
        _NC_CACHE["nc"] = build_nc()
    nc = _NC_CACHE["nc"]

    in_maps = [
        prep_core_inputs(x, qkv_w, qkv_b, proj_w, c) for c in range(NCORES)
    ]
    res = run_bass_kernel_spmd(nc, in_maps, core_ids=list(range(NCORES)))
    outs = res.results

    # v-bias folds into a constant row added to every token: proj_w @ v_bias.
    const_row = proj_w @ qkv_b[2 * D:3 * D] + proj_b
    full = np.empty((B, N, D), np.float32)
    for b in range(B):
        full[b] = outs[2 * b]["out"] + outs[2 * b + 1]["out"] + const_row
    return full


# revision 47
# speedup vs baseline: 1.1638x; 1.1638x over previous
"""Multi-head attention (B=4, N=2048, D=1024, H=16) on 8 TRN2 NeuronCores.

Sharding: core c = (batch b = c // 2, head-group hg = c % 2). Each core:
  - computes Q/K/V for its 8 heads (tensor-parallel slice of qkv_w),
  - runs attention for those heads,
  - computes a partial output projection against its 512 columns of proj_w.
Host sums the two partials per batch and adds biases folded on the host.

Device layouts (all feature-on-partition so that scores come out as
S^T [k, q] with k on partitions — no transposes anywhere):
  xt  [128, 8, 2048]  bf16 : x[b]^T, d = kt*128 + p
  wqk [128, 8, 1024]  bf16 : lhsT for Q (slots 0..3) and K (slots 4..7);
                             slot t covers the head pair (2t, 2t+1), so the
                             PSUM partition j of an output tile = head
                             (2t + j//64), hd = j % 64.
  wv  [128, 8, 512]   bf16 : rhs for V (token-on-partition orientation)
  wp  [128, 4, 1024]  bf16 : lhsT-side contraction layout for the proj
  bqk [128, 8]        f32  : per-feature q/k bias (zero in practice)
  out [2048, 1024]    f32  : partial projection output

Attention per head: S^T = K @ Q^T (k on partitions), P = exp(S^T/8) on ACT
(PSUM->SBUF, bf16), O^T_aug = V_aug.T @ P where V_aug = [V | 1] gives the
softmax denominator as row 64 (in-band).  Normalization multiplies by a
reciprocal broadcast across partitions via a DRAM round-trip DMA.

Softmax skips max-subtraction (scores ~N(0,1); exp never overflows fp32).
V-bias and proj bias are folded host-side: softmax rows sum to 1, so the
V bias contributes exactly proj_w @ v_bias to every output row.

Pair t+1's Q/K projection matmuls are interleaved into pair t's attention
loop to keep the TensorE dense (HAM stays at 2.4 GHz) and hide the QKV
phase inside the ACT-bound attention phase.
"""

import numpy as np
import ml_dtypes

import concourse.tile as tile
from concourse import bacc, mybir
from concourse._compat import with_exitstack

B, N, D, H, HD = 4, 2048, 1024, 16, 64
NCORES = 8
HPC = 8          # heads per core
FPC = HPC * HD   # 512 features per core
KT = 8           # d-contraction tiles of 128
KTT = 16         # key-token tiles of 128
QB = 512         # q-block size
NQB = N // QB
SCALE = HD ** -0.5

F32 = mybir.dt.float32
BF16 = mybir.dt.bfloat16
EXP = mybir.ActivationFunctionType.Exp


@with_exitstack
def _attn_body(ctx, tc, xt_d, wqk_d, wv_d, wp_d, bqk_d, out_d):
    nc = tc.nc

    singles = ctx.enter_context(tc.tile_pool(name="singles", bufs=1))
    evac = ctx.enter_context(tc.tile_pool(name="evac", bufs=4))
    ppool = ctx.enter_context(tc.tile_pool(name="ppool", bufs=4))
    rpool = ctx.enter_context(tc.tile_pool(name="rpool", bufs=4))
    dpool = ctx.enter_context(tc.tile_pool(name="dpool", bufs=8, space="DRAM"))
    ps_s = ctx.enter_context(tc.tile_pool(name="ps_s", bufs=2, space="PSUM"))
    ps_av = ctx.enter_context(tc.tile_pool(name="ps_av", bufs=3, space="PSUM"))
    ps_w = ctx.enter_context(tc.tile_pool(name="ps_w", bufs=1, space="PSUM"))

    # Resident SBUF tensors.  Weights first, then x in token-range chunks so
    # the V projection can start before the full input has landed; wp last
    # (only the projection needs it).
    wv_sb = singles.tile([128, KT, FPC], BF16)
    nc.sync.dma_start(wv_sb, wv_d[:])
    wqk_sb = singles.tile([128, KT, 2 * FPC], BF16)
    nc.sync.dma_start(wqk_sb, wqk_d[:])
    bqk_sb = singles.tile([128, 8], F32)
    nc.sync.dma_start(bqk_sb, bqk_d[:])
    xt_sb = singles.tile([128, KT, N], BF16)
    for c in range(8):
        nc.sync.dma_start(xt_sb[:, :, c * 256:(c + 1) * 256],
                          xt_d[:, :, c * 256:(c + 1) * 256])
    wp_sb = singles.tile([128, 4, D], BF16)
    nc.sync.dma_start(wp_sb, wp_d[:])

    qk_sb = singles.tile([128, 8, N], BF16)          # Q^T slots 0..3, K^T slots 4..7
    v_sb = singles.tile([128, KTT, HPC, HD + 1], BF16)  # V_aug, token-on-partition
    o_sb = singles.tile([128, 4, N], BF16)           # normalized attn out, f-on-part
    nc.vector.memset(v_sb[:, :, :, HD], 1.0)         # the ones column

    def emit_qk(ft, qt, pool, tag):
        """One (ft, qt) group of the Q/K projection: 8 matmuls + bias evac."""
        ps = pool.tile([128, 512], F32, tag=tag, name="qk_ps")
        for kt in range(KT):
            nc.tensor.matmul(
                ps,
                wqk_sb[:, kt, ft * 128:(ft + 1) * 128],
                xt_sb[:, kt, qt * 512:(qt + 1) * 512],
                start=(kt == 0), stop=(kt == KT - 1),
            )
        nc.vector.tensor_scalar_add(
            qk_sb[:, ft, qt * 512:(qt + 1) * 512], ps, bqk_sb[:, ft:ft + 1])

    def emit_v(mt, pool, tag):
        """One token-tile of the V projection: 8 matmuls + strided evac."""
        ps = pool.tile([128, 512], F32, tag=tag, name="v_ps")
        for kt in range(KT):
            nc.tensor.matmul(
                ps,
                xt_sb[:, kt, mt * 128:(mt + 1) * 128],
                wv_sb[:, kt, :],
                start=(kt == 0), stop=(kt == KT - 1),
            )
        nc.vector.tensor_copy(
            v_sb[:, mt, :, 0:HD], ps.rearrange("p (h e) -> p h e", h=HPC))

    def emit_proj(mt, et, pool, tag, on_act=False):
        """One output-projection group: 4 matmuls + evac + store."""
        ps = pool.tile([128, 512], F32, tag=tag, name="pj_ps")
        for t4 in range(4):
            nc.tensor.matmul(
                ps,
                o_sb[:, t4, mt * 128:(mt + 1) * 128],
                wp_sb[:, t4, et * 512:(et + 1) * 512],
                start=(t4 == 0), stop=(t4 == 3),
            )
        ot = evac.tile([128, 512], F32, tag="oevac", name="o_evac")
        if on_act:  # ACT is idle in the tail; DVE may be behind a reciprocal
            nc.scalar.copy(ot, ps)
        else:
            nc.vector.tensor_copy(ot, ps)
        nc.sync.dma_start(
            out_d[mt * 128:(mt + 1) * 128, et * 512:(et + 1) * 512], ot)

    # normalize: o = av * (1/denom); denom = row 64 of avs (ones col).
    # DVE reciprocal runs at 1/8 rate, so a single-partition [1,1024]
    # reciprocal costs ~6.5us: bounce the denominators through DRAM into a
    # [128, 8] layout so the reciprocal uses all 128 lanes (~70ns), and read
    # the result back as a partition-broadcast.  Stage 2 (reciprocal + the
    # multiplies) is deferred one block so the DVE never sits waiting on the
    # DMA chain and blocking the rest of its in-order queue; the multiplies
    # run on the otherwise-idle GpSimd engine, where waiting is harmless.
    def normalize_stage1(t, qb, avs):
        rd = dpool.tile([2, QB], F32, name="d_dram")
        nc.sync.dma_start(rd[0:1, :], avs[HD:HD + 1, 0:QB])
        nc.sync.dma_start(rd[1:2, :], avs[HD:HD + 1, QB:2 * QB])
        d128 = rpool.tile([128, 8], F32, tag="d128", name="d128_t")
        nc.sync.dma_start(
            d128, rd[:].rearrange("two (a p) -> p (two a)", p=128))
        return (t, qb, avs, d128)

    def normalize_stage2(st):
        t, qb, avs, d128 = st
        q0 = qb * QB
        r128 = rpool.tile([128, 8], F32, tag="r128", name="r128_t")
        nc.vector.reciprocal(r128, d128)
        rr = dpool.tile([2, QB], F32, name="r_dram")
        nc.sync.dma_start(
            rr[:].rearrange("two (a p) -> p (two a)", p=128), r128)
        rb = rpool.tile([64, 2 * QB], F32, tag="rb", name="rb_t")
        nc.sync.dma_start(rb[:, 0:QB], rr[0:1, :].partition_broadcast(64))
        nc.sync.dma_start(rb[:, QB:2 * QB],
                          rr[1:2, :].partition_broadcast(64))
        nc.gpsimd.tensor_mul(o_sb[0:64, t, q0:q0 + QB], avs[0:HD, 0:QB],
                             rb[:, 0:QB])
        ot = rpool.tile([64, QB], BF16, tag="otmp", name="o_tmp")
        nc.gpsimd.tensor_mul(ot, avs[0:HD, QB:2 * QB], rb[:, QB:2 * QB])
        nc.gpsimd.dma_start(o_sb[64:128, t, q0:q0 + QB], ot)

    def normalize_direct(t, qb, avs):
        """Pair-3 variant: plain DVE reciprocal (6.5us busy but no DMA
        chain), so o_sb is ready quickly and the projection can stream in
        behind it.  DVE has ~9us of slack per 18us block here (no QK
        evacuations left), so the slow reciprocal just fills idle DVE time
        without delaying the next block's AV evacuations."""
        q0 = qb * QB
        r = rpool.tile([HD + 1, 2 * QB], F32, tag="rdir", name="r_dir")
        nc.vector.reciprocal(r[HD:HD + 1, :], avs[HD:HD + 1, :])
        rb = rpool.tile([64, 2 * QB], F32, tag="rb", name="rb_t")
        rd = dpool.tile([2, QB], F32, name="rdd")
        nc.sync.dma_start(rd[0:1, :], r[HD:HD + 1, 0:QB])
        nc.sync.dma_start(rd[1:2, :], r[HD:HD + 1, QB:2 * QB])
        nc.sync.dma_start(rb[:, 0:QB], rd[0:1, :].partition_broadcast(64))
        nc.sync.dma_start(rb[:, QB:2 * QB], rd[1:2, :].partition_broadcast(64))
        nc.vector.tensor_mul(o_sb[0:64, t, q0:q0 + QB], avs[0:HD, 0:QB],
                             rb[:, 0:QB])
        ot = rpool.tile([64, QB], BF16, tag="otmp", name="o_tmp")
        nc.vector.tensor_mul(ot, avs[0:HD, QB:2 * QB], rb[:, QB:2 * QB])
        nc.gpsimd.dma_start(o_sb[64:128, t, q0:q0 + QB], ot)

    # ---- Prologue: only what pair-0/qb-0 needs soon ----
    # V token-tiles 0..11 (AV(kt) consumes V(mt=kt) one per cycle), K(0,qt)
    # for qt 0,1 (scores kt 0..7) and Q(0,qt=0).
    for mt in range(12):
        emit_v(mt, ps_av, "psAV")
    emit_qk(4, 0, ps_av, "psAV")
    emit_qk(0, 0, ps_av, "psAV")
    emit_qk(4, 1, ps_av, "psAV")

    # Deadline-ordered feeds through the single ps_w slot, rate-limited so
    # the PE never falls behind ACT: pair 0 pops one group every 2 cycles,
    # pairs 1/2 every 4 (8-matmul groups), pair 3 every 2 (4-matmul proj
    # groups, appended as each q-block's normalize lands).
    feeds = [
        [("qk", 4, 2), ("qk", 0, 1), ("qk", 4, 3), ("v", 12), ("v", 13),
         ("v", 14), ("v", 15), ("qk", 0, 2), ("qk", 0, 3)]
        + [("qk", f, qt) for qt in range(4) for f in (1, 5)],
        [("qk", f, qt) for qt in range(4) for f in (2, 6)],
        [("qk", f, qt) for qt in range(4) for f in (3, 7)],
        [],  # pair 3: proj groups appended as q-blocks complete
    ]
    cadence = [2, 4, 4, 2]

    # ---- Attention: one flat software pipeline over (t, qb, kt).
    # AV runs one cycle behind scores/exp so ACT never waits on the PE; each
    # block's normalize is emitted right after its last AV (one cycle into
    # the next block).
    blocks = [(t, qb) for t in range(4) for qb in range(NQB)]
    pending = None  # (t, qb, kt, av_e, av_o, pt, start, fin)
    norm_q = []     # deferred normalize stage-2 states

    def flush_pending():
        nonlocal pending
        if pending is None:
            return
        t, qb, kt, av_e, av_o, pt, st, fin = pending
        nc.tensor.matmul(av_e, v_sb[:, kt, 2 * t, :], pt[:, 0:512],
                         start=st, stop=fin)
        nc.tensor.matmul(av_o, v_sb[:, kt, 2 * t + 1, :],
                         pt[:, 512:1024], start=st, stop=fin)
        if fin:
            avs = rpool.tile([HD + 1, 2 * QB], F32, tag="avs", name="avs_t")
            nc.vector.tensor_copy(avs[:, 0:QB], av_e)
            nc.vector.tensor_copy(avs[:, QB:2 * QB], av_o)
            if t < 3:
                if norm_q:
                    normalize_stage2(norm_q.pop(0))
                norm_q.append(normalize_stage1(t, qb, avs))
            else:
                # pair 3: direct normalize; proj for the PREVIOUS q-block
                # becomes feedable now (its o_sb landed a block ago).
                if qb > 0:
                    feeds[3].extend(
                        [("proj", mt, et)
                         for mt in range(4 * (qb - 1), 4 * qb)
                         for et in range(2)])
                normalize_direct(t, qb, avs)
        pending = None

    for t, qb in blocks:
        q0 = qb * QB
        av_e = ps_av.tile([HD + 1, QB], F32, tag="psAV", name="av_e")
        av_o = ps_av.tile([HD + 1, QB], F32, tag="psAV", name="av_o")
        for kt in range(KTT):
            k0 = kt * 128
            sp = ps_s.tile([128, 1024], F32, tag="psS", name="s_ps")
            # scores S^T for the pair: even head rows 0:64, odd 64:128
            nc.tensor.matmul(
                sp[:, 0:512],
                qk_sb[0:64, 4 + t, k0:k0 + 128],
                qk_sb[0:64, t, q0:q0 + 512],
                start=True, stop=True,
            )
            nc.tensor.matmul(
                sp[:, 512:1024],
                qk_sb[64:128, 4 + t, k0:k0 + 128],
                qk_sb[64:128, t, q0:q0 + 512],
                start=True, stop=True,
            )
            pt = ppool.tile([128, 1024], BF16, tag="pt", name="p_t")
            nc.scalar.activation(pt, sp, EXP, scale=SCALE)
            flush_pending()
            pending = (t, qb, kt, av_e, av_o, pt, kt == 0, kt == KTT - 1)
            if feeds[t] and (qb * KTT + kt) % cadence[t] == 0:
                g = feeds[t].pop(0)
                if g[0] == "qk":
                    emit_qk(g[1], g[2], ps_w, "psW")
                elif g[0] == "v":
                    emit_v(g[1], ps_w, "psW")
                else:
                    emit_proj(g[1], g[2], ps_w, "psW")
    flush_pending()
    while norm_q:
        normalize_stage2(norm_q.pop(0))

    # ---- Tail: remaining projection groups (q-block 2 leftovers via the
    # feed list, then q-block 3's) ----
    for g in feeds[3]:
        emit_proj(g[1], g[2], ps_av, "psAV", on_act=True)
    for mt in range(12, KTT):
        for et in range(2):
            emit_proj(mt, et, ps_av, "psAV", on_act=True)


def build_nc():
    nc = bacc.Bacc()
    xt = nc.declare_dram_parameter("xt", [128, KT, N], BF16, isOutput=False)
    wqk = nc.declare_dram_parameter("wqk", [128, KT, 2 * FPC], BF16, isOutput=False)
    wv = nc.declare_dram_parameter("wv", [128, KT, FPC], BF16, isOutput=False)
    wp = nc.declare_dram_parameter("wp", [128, 4, D], BF16, isOutput=False)
    bqk = nc.declare_dram_parameter("bqk", [128, 8], F32, isOutput=False)
    out = nc.declare_dram_parameter("out", [N, D], F32, isOutput=True)
    with tile.TileContext(nc) as tc:
        _attn_body(tc, xt, wqk, wv, wp, bqk, out)
    nc.finalize()
    return nc


BF = ml_dtypes.bfloat16


def prep_core_inputs(x, qkv_w, qkv_b, proj_w, c):
    """Build the per-core input map (numpy, final SBUF layouts)."""
    b, hg = divmod(c, 2)
    f0 = hg * FPC
    xt = np.ascontiguousarray(x[b].T)                     # [1024, 2048] f32
    xt_sb = xt.reshape(KT, 128, N).transpose(1, 0, 2)     # [128, 8, 2048]
    wq = qkv_w[f0:f0 + FPC]
    wk = qkv_w[D + f0:D + f0 + FPC]
    wqk = np.concatenate([wq, wk], axis=0)                # [1024, 1024]
    wqk_sb = wqk.T.reshape(KT, 128, 2 * FPC).transpose(1, 0, 2)
    wv = qkv_w[2 * D + f0:2 * D + f0 + FPC]               # [512, 1024]
    wv_sb = wv.T.reshape(KT, 128, FPC).transpose(1, 0, 2)
    wp = proj_w[:, f0:f0 + FPC]                           # [1024e, 512f]
    wp_sb = wp.T.reshape(4, 128, D).transpose(1, 0, 2)
    bqk = np.concatenate(
        [qkv_b[f0:f0 + FPC], qkv_b[D + f0:D + f0 + FPC]]).reshape(8, 128).T
    return {
        "xt": np.ascontiguousarray(xt_sb).astype(BF),
        "wqk": np.ascontiguousarray(wqk_sb).astype(BF),
        "wv": np.ascontiguousarray(wv_sb).astype(BF),
        "wp": np.ascontiguousarray(wp_sb).astype(BF),
        "bqk": np.ascontiguousarray(bqk).astype(np.float32),
    }


def expected_core_out(x, qkv_w, qkv_b, proj_w, c):
    """Numpy model of one core's partial output (for sim debugging)."""
    b, hg = divmod(c, 2)
    f0 = hg * FPC
    xb = x[b].astype(np.float32)
    q = xb @ qkv_w[f0:f0 + FPC].T + qkv_b[f0:f0 + FPC]
    k = xb @ qkv_w[D + f0:D + f0 + FPC].T + qkv_b[D + f0:D + f0 + FPC]
    v = xb @ qkv_w[2 * D + f0:2 * D + f0 + FPC].T          # v-bias folded on host
    out = np.zeros((N, D), np.float32)
    for h in range(HPC):
        qs = q[:, h * HD:(h + 1) * HD]
        ks = k[:, h * HD:(h + 1) * HD]
        vs = v[:, h * HD:(h + 1) * HD]
        s = (qs @ ks.T) * SCALE
        p = np.exp(s - s.max(axis=1, keepdims=True))
        p /= p.sum(axis=1, keepdims=True)
        out += (p @ vs) @ proj_w[:, f0 + h * HD:f0 + (h + 1) * HD].T
    return out


_NC_CACHE = {}


def kernel(x, qkv_w, qkv_b, proj_w, proj_b):
    from concourse.bass_utils import run_bass_kernel_spmd

    x = np.asarray(x, dtype=np.float32)
    qkv_w = np.asarray(qkv_w, dtype=np.float32)
    qkv_b = np.asarray(qkv_b, dtype=np.float32)
    proj_w = np.asarray(proj_w, dtype=np.float32)
    proj_b = np.asarray(proj_b, dtype=np.float32)

    if "nc" not in _NC_CACHE:
        _NC_CACHE["nc"] = build_nc()
    nc = _NC_CACHE["nc"]

    in_maps = [
        prep_core_inputs(x, qkv_w, qkv_b, proj_w, c) for c in range(NCORES)
    ]
    res = run_bass_kernel_spmd(nc, in_maps, core_ids=list(range(NCORES)))
    outs = res.results

    # v-bias folds into a constant row added to every token: proj_w @ v_bias.
    const_row = proj_w @ qkv_b[2 * D:3 * D] + proj_b
    full = np.empty((B, N, D), np.float32)
    for b in range(B):
        full[b] = outs[2 * b]["out"] + outs[2 * b + 1]["out"] + const_row
    return full
